# revision 30
# baseline (speedup 1.0000x reference)
"""RGCN (relational GCN) layer on 8 Trainium2 NeuronCores.

out = relu(sum_r mean_{e: rel=r, dst=n} x[src_e] @ W_r + x @ root + bias)

Strategy: dst-node sharding (no collectives). Core c owns dst nodes
[c*6250, (c+1)*6250); every edge lives on its dst's owner core, so each core
computes its output slice independently and the host concatenates.

Device algorithm per core, per dst-tile j (128 dst nodes):
  - dma_gather x[src] rows (bf16) for all edges into G [128, T, 128]
    (x is split in two 25000-row halves so gather indices fit int16).
    Gathers rotate across 4 SWDGE queues (single_packet=False) so the
    descriptor rings never back-pressure the Q7 — the drain runs at the
    random-read rate of all 16 SDMA engines instead of one ring. The
    gather drain (~75K random 256B HBM reads per core, DRAM-row-cycle
    bound) is the kernel's critical resource; everything else overlaps.
  - per relation window r: bps[:, r, :] += G_t^T @ S_t over the window's
    tiles, where S_t[p, q] = w_p * (q == col_p) (col = dst_local,
    w = 1/count) -> the per-(r, dst) *mean*. S tiles are precomputed
    dense on the host, stored fp8 (exact for pow-2 counts, <=4% else),
    streamed per 4-tile group and upcast to bf16 on the otherwise-idle
    DVE. (Building S on DVE from metadata contends with the Q7
    descriptor-ring writes on the shared SBUF port; streaming it bf16
    doubles the stream's SDMA-engine time.)
  - the self/root term needs x_j^T as a transform rhs: host supplies it
    directly as a transposed xselfT tensor, one [128, 512] DMA per group
    into the acc4 slice (no gather, no scatter matmul).
  - one ACT copy bps -> acc4 slice (bf16, [feat, r, jj*128] layout).
  - per group of 4 dst-tiles: out^T = sum_r W_r^T @ acc4_r via 9
    PSUM-accumulated matmuls with N=512 (lhsT = wcat bf16 [d, e]).
  - relu + bias in one ACT op (bias per-partition in transposed layout);
    DMA out in bf16. Host transposes each [e, 4*128] group back to [n, e].

All index preprocessing happens on the host; per-(window, half) tile counts
are maxed over the 8 cores so all cores run the same program (SPMD).
"""
import ml_dtypes
import numpy as np

import concourse.bass as bass
import concourse.mybir as mybir
import concourse.tile as tile
from concourse import bacc
from concourse.bass_utils import run_bass_kernel_spmd

N = 50000
E = 600000
D = 128
R = 8
P = 128
NCORES = 8
NC_NODES = N // NCORES          # 6250
NT = (NC_NODES + P - 1) // P    # 49
H = 25000                       # x half split (int16 index limit)
NR = R + 1                      # relations + self
MAXT = 8                        # <= 1024 idxs per dma_gather call
JG = 4                          # dst-tiles per transform group
NG = (NT + JG - 1) // JG        # 13 groups (last has 1 tile)

F32 = mybir.dt.float32
BF16 = mybir.dt.bfloat16
FP8 = mybir.dt.float8e4
I16 = mybir.dt.int16
BF = ml_dtypes.bfloat16
F8 = ml_dtypes.float8_e4m3


def _preprocess(edge_index, edge_type):
    """Core-invariant tile layout + per-core slot arrays (index data only).

    Slot layout per dst-tile j: [lo tiles (windows r=0..7) | hi tiles].
    """
    src = np.asarray(edge_index[0], dtype=np.int64)
    dst = np.asarray(edge_index[1], dtype=np.int64)
    et = np.asarray(edge_type, dtype=np.int64)

    counts = np.bincount(et * N + dst, minlength=R * N)

    core = dst // NC_NODES
    j = (dst - core * NC_NODES) // P
    half = (src >= H).astype(np.int64)

    key = ((core * NT + j) * R + et) * 2 + half
    cnt = np.bincount(key, minlength=NCORES * NT * R * 2).reshape(NCORES, NT, R, 2)
    tiles = -(-cnt // P)
    Tmax = tiles.max(axis=0)                   # [NT, R, 2]

    T_lo = Tmax[:, :, 0].copy()                # [NT, R]
    T_hi = Tmax[:, :, 1].copy()
    both0 = (T_lo + T_hi) == 0
    T_lo[both0] = 1

    Tlo_tot = T_lo.sum(axis=1)
    Thi_tot = T_hi.sum(axis=1)
    Tj = Tlo_tot + Thi_tot                     # gathered tiles (self separate)

    lo_off = np.zeros((NT, R), dtype=np.int64)
    lo_off[:, 1:] = np.cumsum(T_lo, axis=1)[:, :-1]
    hi_off = np.zeros((NT, R), dtype=np.int64)
    hi_off[:, 1:] = np.cumsum(T_hi, axis=1)[:, :-1]
    hi_off += Tlo_tot[:, None]

    S_tiles = int(Tj.sum())

    jkey = core * NT + j
    order = np.lexsort((half, et, jkey))
    src_s, et_s, core_s, half_s = src[order], et[order], core[order], half[order]
    dst_s = dst[order]
    j_s = (dst_s - core_s * NC_NODES) // P
    col_s = (dst_s - core_s * NC_NODES) % P
    w_s = (1.0 / np.maximum(counts[et_s * N + dst_s], 1)).astype(np.float32)

    tile_base = np.zeros(NT, dtype=np.int64)
    tile_base[1:] = np.cumsum(Tj)[:-1]

    per_core = []
    for c in range(NCORES):
        m = core_s == c
        cs, cj, cr, ccol, cw, chalf = (a[m] for a in (src_s, j_s, et_s, col_s, w_s, half_s))
        gidx = np.zeros(S_tiles * P, dtype=np.int32)
        colv = np.zeros(S_tiles * P, dtype=np.float32)
        wv = np.zeros(S_tiles * P, dtype=np.float32)

        if len(cj):
            wkey = (cj * R + cr) * 2 + chalf
            changed = np.empty(len(wkey), dtype=bool)
            changed[0] = True
            changed[1:] = wkey[1:] != wkey[:-1]
            grp_start = np.maximum.accumulate(np.where(changed, np.arange(len(wkey)), 0))
            pos = np.arange(len(wkey)) - grp_start
            block = np.where(chalf == 0, lo_off[cj, cr], hi_off[cj, cr])
            slot = (tile_base[cj] + block + pos // P) * P + (pos % P)
            gidx[slot] = np.where(chalf == 0, cs, cs - H)
            colv[slot] = ccol
            wv[slot] = cw

        per_core.append({"gidx": gidx, "col": colv, "w": wv})

    layout = {
        "T_lo": T_lo, "T_hi": T_hi, "Tlo_tot": Tlo_tot, "Thi_tot": Thi_tot,
        "Tj": Tj, "lo_off": lo_off, "hi_off": hi_off, "S_tiles": S_tiles,
    }
    return layout, per_core


def _wrap_idxs(flat):
    """dma_gather int16 index layout: idx i at [i%16, i//16], replicated x8."""
    a = np.asarray(flat, dtype=np.int16).reshape(-1, 16).T
    return np.tile(a, (8, 1))


def _build_device_arrays(layout, per_core, x):
    """Per-core arrays: main [128, sum(Tj*8)] int16 idx, sall [128,
    sum(Tj)*128] bf16 dense scatter matrices, and xself [NT*128, 128] bf16
    (the core's dst rows)."""
    Tj, S_tiles = layout["Tj"], layout["S_tiles"]
    x_bf = np.asarray(x, dtype=np.float32).astype(BF)
    out = []
    for c, meta in enumerate(per_core):
        gidx, colv, wv = meta["gidx"], meta["col"], meta["w"]
        cols = []
        base = 0
        for j in range(NT):
            tj = int(Tj[j])
            sl = slice(base * P, (base + tj) * P)
            cols.append(_wrap_idxs(gidx[sl]))
            base += tj
        # dense S tiles: S[t, p, col[t*P+p]] = w[t*P+p] (fp8: 1/count values
        # quantize to <=4% err on the ~19% of counts not a power of two;
        # contributes ~1e-3 to the output rel err)
        sall = np.zeros((S_tiles, P, P), dtype=F8)
        ti = np.repeat(np.arange(S_tiles), P)
        pi = np.tile(np.arange(P), S_tiles)
        sall[ti, pi, colv.astype(np.int64)] = wv.astype(F8)
        sall = np.ascontiguousarray(sall.transpose(1, 0, 2).reshape(P, S_tiles * P))
        # transposed self rows [feat, dst_local] for the root-term rhs
        xsT = np.zeros((P, NG * JG * P), dtype=BF)
        n0 = c * NC_NODES
        xsT[:, :NC_NODES] = x_bf[n0 : n0 + NC_NODES].T
        out.append({
            "main": np.ascontiguousarray(np.concatenate(cols, axis=1), dtype=np.int16),
            "sall": sall,
            "xselfT": xsT,
        })
    return out


def _build_bass(layout):
    T_lo, T_hi = layout["T_lo"], layout["T_hi"]
    Tj, Tlo_tot = layout["Tj"], layout["Tlo_tot"]
    lo_off, hi_off = layout["lo_off"], layout["hi_off"]
    S_tiles = layout["S_tiles"]
    IW = int(sum(int(Tj[j]) * 8 for j in range(NT)))

    nc = bacc.Bacc(None, target_bir_lowering=False, debug=False,
                   num_swdge_queues=4, dynamic_dma_scratch_size=65536)

    xlo = nc.dram_tensor("xlo", [H, D], BF16, kind="ExternalInput")
    xhi = nc.dram_tensor("xhi", [H, D], BF16, kind="ExternalInput")
    xselfT = nc.dram_tensor("xselfT", [P, NG * JG * P], BF16, kind="ExternalInput")
    main = nc.dram_tensor("main", [P, IW], I16, kind="ExternalInput")
    sall = nc.dram_tensor("sall", [P, S_tiles * P], FP8, kind="ExternalInput")
    wcat = nc.dram_tensor("wcat", [P, NR * P], BF16, kind="ExternalInput")
    biasc = nc.dram_tensor("biasc", [P, 1], F32, kind="ExternalInput")
    out_t = nc.dram_tensor("out_t", [NG, P, JG * P], BF16, kind="ExternalOutput")

    qn = [0]

    def next_q():
        q = qn[0]
        qn[0] = (q + 1) % 4
        return q



    with tile.TileContext(nc) as tc:
        with (
            tc.tile_pool(name="const", bufs=1) as cpool,
            tc.tile_pool(name="m", bufs=3) as mpool,
            tc.tile_pool(name="g", bufs=6) as gpool,
            tc.tile_pool(name="acc", bufs=2) as apool,
            tc.tile_pool(name="s8", bufs=3) as spool8,
            tc.tile_pool(name="s", bufs=2) as spool,
            tc.tile_pool(name="o", bufs=2) as opool,
            tc.tile_pool(name="psw", bufs=3, space="PSUM") as pswin,
            tc.tile_pool(name="pso", bufs=2, space="PSUM") as psout,
        ):
            wcat_sb = cpool.tile([P, NR * P], BF16)
            nc.sync.dma_start(wcat_sb[:], wcat[:])
            bias_sb = cpool.tile([P, 1], F32)
            nc.sync.dma_start(bias_sb[:], biasc[:])

            gw = [int(sum(int(Tj[j]) for j in range(g * JG, min((g + 1) * JG, NT))))
                  for g in range(NG)]

            iw = 0
            sw = 0
            for g in range(NG):
                js = [j for j in range(g * JG, min((g + 1) * JG, NT))]
                g0 = g * JG
                acc4 = apool.tile([P, NR, JG * P], BF16, tag="acc")
                # gather-index slice for the group (1KB rows)
                main_sb = mpool.tile([P, gw[g] * 8], I16, tag="m")
                nc.sync.dma_start(main_sb[:], main[:, iw : iw + gw[g] * 8])
                # root-term rhs: transposed self rows, one DMA per group
                nc.scalar.dma_start(acc4[:, R, :],
                                    xselfT[:, g * JG * P : (g + 1) * JG * P])
                # scatter matrices for the whole group: one fp8 DMA (8KB rows)
                # + DVE upcast to bf16 (DVE is otherwise idle)
                sld8 = spool8.tile([P, gw[g] * P], FP8, tag="s8")
                nc.sync.dma_start(sld8[:], sall[:, sw : sw + gw[g] * P])
                sld = spool.tile([P, gw[g] * P], BF16, tag="s")
                nc.vector.tensor_copy(sld[:], sld8[:])
                st = 0
                giw = 0
                for j in js:
                    jj = j - g * JG
                    tj = int(Tj[j])
                    tlo = int(Tlo_tot[j])
                    thi = tj - tlo

                    def gath(out_ap, in_ap, idxs_ap, n):
                        nc.gpsimd.dma_gather(
                            out_ap=out_ap, in_ap=in_ap, idxs_ap=idxs_ap,
                            num_idxs=n, num_idxs_reg=n, elem_size=P,
                            single_packet=False, queue_num=next_q(),
                        )

                    G = gpool.tile([P, tj, P], BF16, tag="g")
                    for t0 in range(0, tlo, MAXT):
                        t1 = min(t0 + MAXT, tlo)
                        gath(G[:, t0:t1, :], xlo[:],
                             main_sb[:, giw + t0 * 8 : giw + t1 * 8], (t1 - t0) * P)
                    for t0 in range(0, thi, MAXT):
                        t1 = min(t0 + MAXT, thi)
                        gath(G[:, tlo + t0 : tlo + t1, :], xhi[:],
                             main_sb[:, giw + (tlo + t0) * 8 : giw + (tlo + t1) * 8],
                             (t1 - t0) * P)

                    bps = pswin.tile([P, R, P], F32, tag="psw")
                    for r in range(R):
                        blocks = [int(lo_off[j, r]) + t for t in range(int(T_lo[j, r]))] + \
                                 [int(hi_off[j, r]) + t for t in range(int(T_hi[j, r]))]
                        for k, b in enumerate(blocks):
                            nc.tensor.matmul(
                                bps[:, r, :], lhsT=G[:, b, :],
                                rhs=sld[:, (st + b) * P : (st + b + 1) * P],
                                start=(k == 0), stop=(k == len(blocks) - 1),
                            )
                    nc.scalar.copy(acc4[:, 0:R, jj * P : (jj + 1) * P], bps[:])

                    iw += tj * 8
                    giw += tj * 8
                    st += tj

                sw += gw[g] * P
                ops = psout.tile([P, JG * P], F32, tag="pso")
                for r in range(NR):
                    nc.tensor.matmul(
                        ops[:], lhsT=wcat_sb[:, r * P : (r + 1) * P],
                        rhs=acc4[:, r, :],
                        start=(r == 0), stop=(r == NR - 1),
                    )
                osb = opool.tile([P, JG * P], BF16, tag="o")
                nc.scalar.activation(
                    osb[:], ops[:], mybir.ActivationFunctionType.Relu,
                    bias=bias_sb[:, 0:1],
                )
                nc.sync.dma_start(out_t[g], osb[:])

    nc.compile()
    return nc


def _host_inputs(inputs):
    x = np.ascontiguousarray(np.asarray(inputs["x"]), dtype=np.float32)
    layout, per_core = _preprocess(np.asarray(inputs["edge_index"]),
                                   np.asarray(inputs["edge_type"]))
    dev = _build_device_arrays(layout, per_core, x)

    weight = np.asarray(inputs["weight"], np.float32)
    root = np.asarray(inputs["root"], np.float32)
    wcat = np.ascontiguousarray(
        np.concatenate([weight[r] for r in range(R)] + [root], axis=1)
    ).astype(BF)
    biasc = np.ascontiguousarray(np.asarray(inputs["bias"], np.float32)[:, None])
    x_bf = x.astype(BF)
    in_maps = [
        {"xlo": np.ascontiguousarray(x_bf[:H]), "xhi": np.ascontiguousarray(x_bf[H:]),
         "xselfT": dev[c]["xselfT"], "main": dev[c]["main"], "sall": dev[c]["sall"],
         "wcat": wcat, "biasc": biasc}
        for c in range(NCORES)
    ]
    return layout, in_maps


def kernel(x, edge_index, edge_type, weight, root, bias, _trace=False):
    inputs = {"x": x, "edge_index": edge_index, "edge_type": edge_type,
              "weight": weight, "root": root, "bias": bias}
    layout, in_maps = _host_inputs(inputs)
    nc = _build_bass(layout)
    res = run_bass_kernel_spmd(nc, in_maps, core_ids=list(range(NCORES)), trace=_trace)

    outs = []
    for c in range(NCORES):
        o = res.results[c]["out_t"].transpose(0, 2, 1).reshape(NG * JG * P, D)
        outs.append(o[:NC_NODES])
    full = np.ascontiguousarray(np.concatenate(outs, axis=0), dtype=np.float32)
    if _trace:
        return full, res
    return full


# revision 34
# speedup vs baseline: 1.1609x; 1.1609x over previous
"""RGCN (relational GCN) layer on 8 Trainium2 NeuronCores.

out = relu(sum_r mean_{e: rel=r, dst=n} x[src_e] @ W_r + x @ root + bias)

Strategy: dst-node sharding (no collectives). Core c owns dst nodes
[c*6250, (c+1)*6250); every edge lives on its dst's owner core, so each core
computes its output slice independently and the host concatenates.

Device algorithm per core, per dst-tile j (128 dst nodes):
  - dma_gather x[src] rows (bf16) for all edges into G [128, T, 128]
    (x is split in two 25000-row halves so gather indices fit int16).
    Gathers rotate across 4 SWDGE queues (single_packet=False) so the
    descriptor rings never back-pressure the Q7 — the drain runs at the
    random-read rate of all 16 SDMA engines instead of one ring. The
    gather drain (~75K random 256B HBM reads per core, DRAM-row-cycle
    bound) is the kernel's critical resource; everything else overlaps.
  - per relation window r: bps[:, r, :] += G_t^T @ S_t over the window's
    tiles, where S_t[p, q] = w_p * (q == col_p) (col = dst_local,
    w = 1/count) -> the per-(r, dst) *mean*. S tiles are precomputed
    dense on the host, stored fp8 (exact for pow-2 counts, <=4% else),
    streamed per 4-tile group and upcast to bf16 on the otherwise-idle
    DVE. (Building S on DVE from metadata contends with the Q7
    descriptor-ring writes on the shared SBUF port; streaming it bf16
    doubles the stream's SDMA-engine time.)
  - the self/root term needs x_j^T as a transform rhs: host supplies it
    directly as a transposed xselfT tensor, one [128, 512] DMA per group
    into the acc4 slice (no gather, no scatter matmul).
  - one ACT copy bps -> acc4 slice (bf16, [feat, r, jj*128] layout).
  - per group of 4 dst-tiles: out^T = sum_r W_r^T @ acc4_r via 9
    PSUM-accumulated matmuls with N=512 (lhsT = wcat bf16 [d, e]).
  - relu + bias in one ACT op (bias per-partition in transposed layout);
    DMA out in bf16. Host transposes each [e, 4*128] group back to [n, e].

All index preprocessing happens on the host; per-(window, half) tile counts
are maxed over the 8 cores so all cores run the same program (SPMD).
"""
import ml_dtypes
import numpy as np

import concourse.bass as bass
import concourse.mybir as mybir
import concourse.tile as tile
from concourse import bacc
from concourse.bass_utils import run_bass_kernel_spmd

N = 50000
E = 600000
D = 128
R = 8
P = 128
NCORES = 8
NC_NODES = N // NCORES          # 6250
NT = (NC_NODES + P - 1) // P    # 49
H = 25000                       # x half split (int16 index limit)
NR = R + 1                      # relations + self
MAXT = 8                        # <= 1024 idxs per dma_gather call
JG = 4                          # dst-tiles per transform group
NG = (NT + JG - 1) // JG        # 13 groups (last has 1 tile)

F32 = mybir.dt.float32
BF16 = mybir.dt.bfloat16
FP8 = mybir.dt.float8e4
I16 = mybir.dt.int16
BF = ml_dtypes.bfloat16
F8 = ml_dtypes.float8_e4m3


def _preprocess(edge_index, edge_type):
    """Core-invariant tile layout + per-core slot arrays (index data only).

    Slot layout per dst-tile j: [lo tiles (windows r=0..7) | hi tiles].
    """
    src = np.asarray(edge_index[0], dtype=np.int64)
    dst = np.asarray(edge_index[1], dtype=np.int64)
    et = np.asarray(edge_type, dtype=np.int64)

    counts = np.bincount(et * N + dst, minlength=R * N)

    core = dst // NC_NODES
    j = (dst - core * NC_NODES) // P
    half = (src >= H).astype(np.int64)

    key = ((core * NT + j) * R + et) * 2 + half
    cnt = np.bincount(key, minlength=NCORES * NT * R * 2).reshape(NCORES, NT, R, 2)
    tiles = -(-cnt // P)
    Tmax = tiles.max(axis=0)                   # [NT, R, 2]

    T_lo = Tmax[:, :, 0].copy()                # [NT, R]
    T_hi = Tmax[:, :, 1].copy()
    both0 = (T_lo + T_hi) == 0
    T_lo[both0] = 1

    Tlo_tot = T_lo.sum(axis=1)
    Thi_tot = T_hi.sum(axis=1)
    Tj = Tlo_tot + Thi_tot                     # gathered tiles (self separate)

    lo_off = np.zeros((NT, R), dtype=np.int64)
    lo_off[:, 1:] = np.cumsum(T_lo, axis=1)[:, :-1]
    hi_off = np.zeros((NT, R), dtype=np.int64)
    hi_off[:, 1:] = np.cumsum(T_hi, axis=1)[:, :-1]
    hi_off += Tlo_tot[:, None]

    S_tiles = int(Tj.sum())

    jkey = core * NT + j
    order = np.lexsort((half, et, jkey))
    src_s, et_s, core_s, half_s = src[order], et[order], core[order], half[order]
    dst_s = dst[order]
    j_s = (dst_s - core_s * NC_NODES) // P
    col_s = (dst_s - core_s * NC_NODES) % P
    w_s = (1.0 / np.maximum(counts[et_s * N + dst_s], 1)).astype(np.float32)

    tile_base = np.zeros(NT, dtype=np.int64)
    tile_base[1:] = np.cumsum(Tj)[:-1]

    per_core = []
    for c in range(NCORES):
        m = core_s == c
        cs, cj, cr, ccol, cw, chalf = (a[m] for a in (src_s, j_s, et_s, col_s, w_s, half_s))
        gidx = np.zeros(S_tiles * P, dtype=np.int32)
        colv = np.zeros(S_tiles * P, dtype=np.float32)
        wv = np.zeros(S_tiles * P, dtype=np.float32)

        if len(cj):
            wkey = (cj * R + cr) * 2 + chalf
            changed = np.empty(len(wkey), dtype=bool)
            changed[0] = True
            changed[1:] = wkey[1:] != wkey[:-1]
            grp_start = np.maximum.accumulate(np.where(changed, np.arange(len(wkey)), 0))
            pos = np.arange(len(wkey)) - grp_start
            block = np.where(chalf == 0, lo_off[cj, cr], hi_off[cj, cr])
            slot = (tile_base[cj] + block + pos // P) * P + (pos % P)
            gidx[slot] = np.where(chalf == 0, cs, cs - H)
            colv[slot] = ccol
            wv[slot] = cw

        per_core.append({"gidx": gidx, "col": colv, "w": wv})

    layout = {
        "T_lo": T_lo, "T_hi": T_hi, "Tlo_tot": Tlo_tot, "Thi_tot": Thi_tot,
        "Tj": Tj, "lo_off": lo_off, "hi_off": hi_off, "S_tiles": S_tiles,
    }
    return layout, per_core


def _wrap_idxs(flat):
    """dma_gather int16 index layout: idx i at [i%16, i//16], replicated x8."""
    a = np.asarray(flat, dtype=np.int16).reshape(-1, 16).T
    return np.tile(a, (8, 1))


def _build_device_arrays(layout, per_core, x):
    """Per-core arrays: main [128, sum(Tj*8)] int16 idx, sall [128,
    sum(Tj)*128] bf16 dense scatter matrices, and xself [NT*128, 128] bf16
    (the core's dst rows)."""
    Tj, S_tiles = layout["Tj"], layout["S_tiles"]
    x_bf = np.asarray(x, dtype=np.float32).astype(BF)
    out = []
    for c, meta in enumerate(per_core):
        gidx, colv, wv = meta["gidx"], meta["col"], meta["w"]
        cols = []
        base = 0
        for j in range(NT):
            tj = int(Tj[j])
            sl = slice(base * P, (base + tj) * P)
            cols.append(_wrap_idxs(gidx[sl]))
            base += tj
        # dense S tiles: S[t, p, col[t*P+p]] = w[t*P+p] (fp8: 1/count values
        # quantize to <=4% err on the ~19% of counts not a power of two;
        # contributes ~1e-3 to the output rel err)
        sall = np.zeros((S_tiles, P, P), dtype=F8)
        ti = np.repeat(np.arange(S_tiles), P)
        pi = np.tile(np.arange(P), S_tiles)
        sall[ti, pi, colv.astype(np.int64)] = wv.astype(F8)
        sall = np.ascontiguousarray(sall.transpose(1, 0, 2).reshape(P, S_tiles * P))
        # transposed self rows [feat, dst_local] for the root-term rhs
        xsT = np.zeros((P, NG * JG * P), dtype=BF)
        n0 = c * NC_NODES
        xsT[:, :NC_NODES] = x_bf[n0 : n0 + NC_NODES].T
        out.append({
            "main": np.ascontiguousarray(np.concatenate(cols, axis=1), dtype=np.int16),
            "sall": sall,
            "xselfT": xsT,
        })
    return out


def _build_bass(layout):
    T_lo, T_hi = layout["T_lo"], layout["T_hi"]
    Tj, Tlo_tot = layout["Tj"], layout["Tlo_tot"]
    lo_off, hi_off = layout["lo_off"], layout["hi_off"]
    S_tiles = layout["S_tiles"]
    IW = int(sum(int(Tj[j]) * 8 for j in range(NT)))

    nc = bacc.Bacc(None, target_bir_lowering=False, debug=False,
                   num_swdge_queues=4, dynamic_dma_scratch_size=65536)

    xlo = nc.dram_tensor("xlo", [H, D], BF16, kind="ExternalInput")
    xhi = nc.dram_tensor("xhi", [H, D], BF16, kind="ExternalInput")
    xselfT = nc.dram_tensor("xselfT", [P, NG * JG * P], BF16, kind="ExternalInput")
    main = nc.dram_tensor("main", [P, IW], I16, kind="ExternalInput")
    sall = nc.dram_tensor("sall", [P, S_tiles * P], FP8, kind="ExternalInput")
    wcat = nc.dram_tensor("wcat", [P, NR * P], BF16, kind="ExternalInput")
    biasc = nc.dram_tensor("biasc", [P, 1], F32, kind="ExternalInput")
    out_t = nc.dram_tensor("out_t", [NG, P, JG * P], BF16, kind="ExternalOutput")

    qn = [0]

    def next_q():
        q = qn[0]
        qn[0] = (q + 1) % 4
        return q



    with tile.TileContext(nc) as tc:
        with (
            tc.tile_pool(name="const", bufs=1) as cpool,
            tc.tile_pool(name="g", bufs=6) as gpool,
            tc.tile_pool(name="acc", bufs=2) as apool,
            tc.tile_pool(name="s8", bufs=3) as spool8,
            tc.tile_pool(name="s", bufs=2) as spool,
            tc.tile_pool(name="o", bufs=2) as opool,
            tc.tile_pool(name="psw", bufs=3, space="PSUM") as pswin,
            tc.tile_pool(name="pso", bufs=2, space="PSUM") as psout,
        ):
            wcat_sb = cpool.tile([P, NR * P], BF16)
            nc.sync.dma_start(wcat_sb[:], wcat[:])
            bias_sb = cpool.tile([P, 1], F32)
            nc.sync.dma_start(bias_sb[:], biasc[:])
            # whole int16 gather-index table resident (12.5 KB/partition) —
            # per-group slices would queue behind the big S transfers in the
            # sync HWDGE FIFO and stall the gather issue.
            main_sb = cpool.tile([P, IW], I16)
            nc.sync.dma_start(main_sb[:], main[:])

            gw = [int(sum(int(Tj[j]) for j in range(g * JG, min((g + 1) * JG, NT))))
                  for g in range(NG)]

            iw = 0
            sw = 0
            for g in range(NG):
                js = [j for j in range(g * JG, min((g + 1) * JG, NT))]
                acc4 = apool.tile([P, NR, JG * P], BF16, tag="acc")
                # root-term rhs: transposed self rows, one DMA per group
                nc.scalar.dma_start(acc4[:, R, :],
                                    xselfT[:, g * JG * P : (g + 1) * JG * P])
                # scatter matrices for the whole group: one fp8 DMA (8KB rows)
                # + DVE upcast to bf16 (DVE is otherwise idle)
                sld8 = spool8.tile([P, gw[g] * P], FP8, tag="s8")
                nc.sync.dma_start(sld8[:], sall[:, sw : sw + gw[g] * P])
                sld = spool.tile([P, gw[g] * P], BF16, tag="s")
                nc.vector.tensor_copy(sld[:], sld8[:])
                st = 0
                for j in js:
                    jj = j - g * JG
                    tj = int(Tj[j])
                    tlo = int(Tlo_tot[j])
                    thi = tj - tlo

                    def gath(out_ap, in_ap, idxs_ap, n):
                        nc.gpsimd.dma_gather(
                            out_ap=out_ap, in_ap=in_ap, idxs_ap=idxs_ap,
                            num_idxs=n, num_idxs_reg=n, elem_size=P,
                            single_packet=False, queue_num=next_q(),
                        )

                    G = gpool.tile([P, tj, P], BF16, tag="g")
                    for t0 in range(0, tlo, MAXT):
                        t1 = min(t0 + MAXT, tlo)
                        gath(G[:, t0:t1, :], xlo[:],
                             main_sb[:, iw + t0 * 8 : iw + t1 * 8], (t1 - t0) * P)
                    for t0 in range(0, thi, MAXT):
                        t1 = min(t0 + MAXT, thi)
                        gath(G[:, tlo + t0 : tlo + t1, :], xhi[:],
                             main_sb[:, iw + (tlo + t0) * 8 : iw + (tlo + t1) * 8],
                             (t1 - t0) * P)

                    bps = pswin.tile([P, R, P], F32, tag="psw")
                    for r in range(R):
                        blocks = [int(lo_off[j, r]) + t for t in range(int(T_lo[j, r]))] + \
                                 [int(hi_off[j, r]) + t for t in range(int(T_hi[j, r]))]
                        for k, b in enumerate(blocks):
                            nc.tensor.matmul(
                                bps[:, r, :], lhsT=G[:, b, :],
                                rhs=sld[:, (st + b) * P : (st + b + 1) * P],
                                start=(k == 0), stop=(k == len(blocks) - 1),
                            )
                    nc.scalar.copy(acc4[:, 0:R, jj * P : (jj + 1) * P], bps[:])

                    iw += tj * 8
                    st += tj

                sw += gw[g] * P
                ops = psout.tile([P, JG * P], F32, tag="pso")
                for r in range(NR):
                    nc.tensor.matmul(
                        ops[:], lhsT=wcat_sb[:, r * P : (r + 1) * P],
                        rhs=acc4[:, r, :],
                        start=(r == 0), stop=(r == NR - 1),
                    )
                osb = opool.tile([P, JG * P], BF16, tag="o")
                nc.scalar.activation(
                    osb[:], ops[:], mybir.ActivationFunctionType.Relu,
                    bias=bias_sb[:, 0:1],
                )
                nc.sync.dma_start(out_t[g], osb[:])

    nc.compile()
    return nc


def _host_inputs(inputs):
    x = np.ascontiguousarray(np.asarray(inputs["x"]), dtype=np.float32)
    layout, per_core = _preprocess(np.asarray(inputs["edge_index"]),
                                   np.asarray(inputs["edge_type"]))
    dev = _build_device_arrays(layout, per_core, x)

    weight = np.asarray(inputs["weight"], np.float32)
    root = np.asarray(inputs["root"], np.float32)
    wcat = np.ascontiguousarray(
        np.concatenate([weight[r] for r in range(R)] + [root], axis=1)
    ).astype(BF)
    biasc = np.ascontiguousarray(np.asarray(inputs["bias"], np.float32)[:, None])
    x_bf = x.astype(BF)
    in_maps = [
        {"xlo": np.ascontiguousarray(x_bf[:H]), "xhi": np.ascontiguousarray(x_bf[H:]),
         "xselfT": dev[c]["xselfT"], "main": dev[c]["main"], "sall": dev[c]["sall"],
         "wcat": wcat, "biasc": biasc}
        for c in range(NCORES)
    ]
    return layout, in_maps


def kernel(x, edge_index, edge_type, weight, root, bias, _trace=False):
    inputs = {"x": x, "edge_index": edge_index, "edge_type": edge_type,
              "weight": weight, "root": root, "bias": bias}
    layout, in_maps = _host_inputs(inputs)
    nc = _build_bass(layout)
    res = run_bass_kernel_spmd(nc, in_maps, core_ids=list(range(NCORES)), trace=_trace)

    outs = []
    for c in range(NCORES):
        o = res.results[c]["out_t"].transpose(0, 2, 1).reshape(NG * JG * P, D)
        outs.append(o[:NC_NODES])
    full = np.ascontiguousarray(np.concatenate(outs, axis=0), dtype=np.float32)
    if _trace:
        return full, res
    return full


# revision 35
# speedup vs baseline: 1.2504x; 1.0771x over previous
"""RGCN (relational GCN) layer on 8 Trainium2 NeuronCores.

out = relu(sum_r mean_{e: rel=r, dst=n} x[src_e] @ W_r + x @ root + bias)

Strategy: dst-node sharding (no collectives). Core c owns dst nodes
[c*6250, (c+1)*6250); every edge lives on its dst's owner core, so each core
computes its output slice independently and the host concatenates.

Device algorithm per core, per dst-tile j (128 dst nodes):
  - dma_gather x[src] rows (bf16) for all edges into G [128, T, 128]
    (x is split in two 25000-row halves so gather indices fit int16).
    Gathers rotate across 4 SWDGE queues (single_packet=False) so the
    descriptor rings never back-pressure the Q7 — the drain runs at the
    random-read rate of all 16 SDMA engines instead of one ring. The
    gather drain (~75K random 256B HBM reads per core, DRAM-row-cycle
    bound) is the kernel's critical resource; everything else overlaps.
  - per relation window r: bps[:, r, :] += G_t^T @ S_t over the window's
    tiles, where S_t[p, q] = w_p * (q == col_p) (col = dst_local,
    w = 1/count) -> the per-(r, dst) *mean*. S tiles are precomputed
    dense on the host, stored fp8 (exact for pow-2 counts, <=4% else),
    streamed per 4-tile group and upcast to bf16 on the otherwise-idle
    DVE. (Building S on DVE from metadata contends with the Q7
    descriptor-ring writes on the shared SBUF port; streaming it bf16
    doubles the stream's SDMA-engine time.)
  - the self/root term needs x_j^T as a transform rhs: host supplies it
    directly as a transposed xselfT tensor, one [128, 512] DMA per group
    into the acc4 slice (no gather, no scatter matmul).
  - one ACT copy bps -> acc4 slice (bf16, [feat, r, jj*128] layout).
  - per group of 4 dst-tiles: out^T = sum_r W_r^T @ acc4_r via 9
    PSUM-accumulated matmuls with N=512 (lhsT = wcat bf16 [d, e]).
  - relu + bias in one ACT op (bias per-partition in transposed layout);
    DMA out in bf16. Host transposes each [e, 4*128] group back to [n, e].

All index preprocessing happens on the host; per-(window, half) tile counts
are maxed over the 8 cores so all cores run the same program (SPMD).
"""
import ml_dtypes
import numpy as np

import concourse.bass as bass
import concourse.mybir as mybir
import concourse.tile as tile
from concourse import bacc
from concourse.bass_utils import run_bass_kernel_spmd

N = 50000
E = 600000
D = 128
R = 8
P = 128
NCORES = 8
NC_NODES = N // NCORES          # 6250
NT = (NC_NODES + P - 1) // P    # 49
H = 25000                       # x half split (int16 index limit)
NR = R + 1                      # relations + self
MAXT = 8                        # <= 1024 idxs per dma_gather call
JG = 4                          # dst-tiles per transform group
NG = (NT + JG - 1) // JG        # 13 groups (last has 1 tile)

F32 = mybir.dt.float32
BF16 = mybir.dt.bfloat16
FP8 = mybir.dt.float8e4
I16 = mybir.dt.int16
BF = ml_dtypes.bfloat16
F8 = ml_dtypes.float8_e4m3


def _preprocess(edge_index, edge_type):
    """Core-invariant tile layout + per-core slot arrays (index data only).

    Slot layout per dst-tile j: [lo tiles (windows r=0..7) | hi tiles].
    """
    src = np.asarray(edge_index[0], dtype=np.int64)
    dst = np.asarray(edge_index[1], dtype=np.int64)
    et = np.asarray(edge_type, dtype=np.int64)

    counts = np.bincount(et * N + dst, minlength=R * N)

    core = dst // NC_NODES
    j = (dst - core * NC_NODES) // P
    half = (src >= H).astype(np.int64)

    key = ((core * NT + j) * R + et) * 2 + half
    cnt = np.bincount(key, minlength=NCORES * NT * R * 2).reshape(NCORES, NT, R, 2)
    tiles = -(-cnt // P)
    Tmax = tiles.max(axis=0)                   # [NT, R, 2]

    T_lo = Tmax[:, :, 0].copy()                # [NT, R]
    T_hi = Tmax[:, :, 1].copy()
    both0 = (T_lo + T_hi) == 0
    T_lo[both0] = 1

    Tlo_tot = T_lo.sum(axis=1)
    Thi_tot = T_hi.sum(axis=1)
    Tj = Tlo_tot + Thi_tot                     # gathered tiles (self separate)

    lo_off = np.zeros((NT, R), dtype=np.int64)
    lo_off[:, 1:] = np.cumsum(T_lo, axis=1)[:, :-1]
    hi_off = np.zeros((NT, R), dtype=np.int64)
    hi_off[:, 1:] = np.cumsum(T_hi, axis=1)[:, :-1]
    hi_off += Tlo_tot[:, None]

    S_tiles = int(Tj.sum())

    jkey = core * NT + j
    order = np.lexsort((half, et, jkey))
    src_s, et_s, core_s, half_s = src[order], et[order], core[order], half[order]
    dst_s = dst[order]
    j_s = (dst_s - core_s * NC_NODES) // P
    col_s = (dst_s - core_s * NC_NODES) % P
    w_s = (1.0 / np.maximum(counts[et_s * N + dst_s], 1)).astype(np.float32)

    tile_base = np.zeros(NT, dtype=np.int64)
    tile_base[1:] = np.cumsum(Tj)[:-1]

    per_core = []
    for c in range(NCORES):
        m = core_s == c
        cs, cj, cr, ccol, cw, chalf = (a[m] for a in (src_s, j_s, et_s, col_s, w_s, half_s))
        gidx = np.zeros(S_tiles * P, dtype=np.int32)
        colv = np.zeros(S_tiles * P, dtype=np.float32)
        wv = np.zeros(S_tiles * P, dtype=np.float32)

        if len(cj):
            wkey = (cj * R + cr) * 2 + chalf
            changed = np.empty(len(wkey), dtype=bool)
            changed[0] = True
            changed[1:] = wkey[1:] != wkey[:-1]
            grp_start = np.maximum.accumulate(np.where(changed, np.arange(len(wkey)), 0))
            pos = np.arange(len(wkey)) - grp_start
            block = np.where(chalf == 0, lo_off[cj, cr], hi_off[cj, cr])
            slot = (tile_base[cj] + block + pos // P) * P + (pos % P)
            gidx[slot] = np.where(chalf == 0, cs, cs - H)
            colv[slot] = ccol
            wv[slot] = cw

        per_core.append({"gidx": gidx, "col": colv, "w": wv})

    layout = {
        "T_lo": T_lo, "T_hi": T_hi, "Tlo_tot": Tlo_tot, "Thi_tot": Thi_tot,
        "Tj": Tj, "lo_off": lo_off, "hi_off": hi_off, "S_tiles": S_tiles,
    }
    return layout, per_core


def _wrap_idxs(flat):
    """dma_gather int16 index layout: idx i at [i%16, i//16], replicated x8."""
    a = np.asarray(flat, dtype=np.int16).reshape(-1, 16).T
    return np.tile(a, (8, 1))


def _build_device_arrays(layout, per_core, x):
    """Per-core arrays: main [128, sum(Tj*8)] int16 idx, sall [128,
    sum(Tj)*128] bf16 dense scatter matrices, and xself [NT*128, 128] bf16
    (the core's dst rows)."""
    Tj, S_tiles = layout["Tj"], layout["S_tiles"]
    x_bf = np.asarray(x, dtype=np.float32).astype(BF)
    out = []
    for c, meta in enumerate(per_core):
        gidx, colv, wv = meta["gidx"], meta["col"], meta["w"]
        cols = []
        base = 0
        for j in range(NT):
            tj = int(Tj[j])
            sl = slice(base * P, (base + tj) * P)
            cols.append(_wrap_idxs(gidx[sl]))
            base += tj
        # dense S tiles: S[t, p, col[t*P+p]] = w[t*P+p] (fp8: 1/count values
        # quantize to <=4% err on the ~19% of counts not a power of two;
        # contributes ~1e-3 to the output rel err)
        sall = np.zeros((S_tiles, P, P), dtype=F8)
        ti = np.repeat(np.arange(S_tiles), P)
        pi = np.tile(np.arange(P), S_tiles)
        sall[ti, pi, colv.astype(np.int64)] = wv.astype(F8)
        sall = np.ascontiguousarray(sall.transpose(1, 0, 2).reshape(P, S_tiles * P))
        # transposed self rows [feat, dst_local] for the root-term rhs
        xsT = np.zeros((P, NG * JG * P), dtype=BF)
        n0 = c * NC_NODES
        xsT[:, :NC_NODES] = x_bf[n0 : n0 + NC_NODES].T
        out.append({
            "main": np.ascontiguousarray(np.concatenate(cols, axis=1), dtype=np.int16),
            "sall": sall,
            "xselfT": xsT,
        })
    return out


def _build_bass(layout):
    T_lo, T_hi = layout["T_lo"], layout["T_hi"]
    Tj, Tlo_tot = layout["Tj"], layout["Tlo_tot"]
    lo_off, hi_off = layout["lo_off"], layout["hi_off"]
    S_tiles = layout["S_tiles"]
    IW = int(sum(int(Tj[j]) * 8 for j in range(NT)))

    nc = bacc.Bacc(None, target_bir_lowering=False, debug=False,
                   num_swdge_queues=4, dynamic_dma_scratch_size=65536)

    xlo = nc.dram_tensor("xlo", [H, D], BF16, kind="ExternalInput")
    xhi = nc.dram_tensor("xhi", [H, D], BF16, kind="ExternalInput")
    xselfT = nc.dram_tensor("xselfT", [P, NG * JG * P], BF16, kind="ExternalInput")
    main = nc.dram_tensor("main", [P, IW], I16, kind="ExternalInput")
    sall = nc.dram_tensor("sall", [P, S_tiles * P], FP8, kind="ExternalInput")
    wcat = nc.dram_tensor("wcat", [P, NR * P], BF16, kind="ExternalInput")
    biasc = nc.dram_tensor("biasc", [P, 1], F32, kind="ExternalInput")
    out_t = nc.dram_tensor("out_t", [NG, P, JG * P], BF16, kind="ExternalOutput")

    qn = [0]

    def next_q():
        q = qn[0]
        qn[0] = (q + 1) % 4
        return q



    with tile.TileContext(nc) as tc:
        with (
            tc.tile_pool(name="const", bufs=1) as cpool,
            tc.tile_pool(name="g", bufs=6) as gpool,
            tc.tile_pool(name="acc", bufs=2) as apool,
            tc.tile_pool(name="s8", bufs=2) as spool8,
            tc.tile_pool(name="s", bufs=2) as spool,
            tc.tile_pool(name="o", bufs=2) as opool,
            tc.tile_pool(name="psw", bufs=3, space="PSUM") as pswin,
            tc.tile_pool(name="pso", bufs=2, space="PSUM") as psout,
        ):
            wcat_sb = cpool.tile([P, NR * P], BF16)
            nc.sync.dma_start(wcat_sb[:], wcat[:])
            bias_sb = cpool.tile([P, 1], F32)
            nc.sync.dma_start(bias_sb[:], biasc[:])
            # whole int16 gather-index table resident (12.5 KB/partition) —
            # per-group slices would queue behind the big S transfers in the
            # sync HWDGE FIFO and stall the gather issue.
            main_sb = cpool.tile([P, IW], I16)
            nc.sync.dma_start(main_sb[:], main[:])

            gw = [int(sum(int(Tj[j]) for j in range(g * JG, min((g + 1) * JG, NT))))
                  for g in range(NG)]

            iw = 0
            sw = 0
            for g in range(NG):
                js = [j for j in range(g * JG, min((g + 1) * JG, NT))]
                acc4 = apool.tile([P, NR, JG * P], BF16, tag="acc")
                # root-term rhs: transposed self rows, one DMA per group
                nc.scalar.dma_start(acc4[:, R, :],
                                    xselfT[:, g * JG * P : (g + 1) * JG * P])
                # scatter matrices for the whole group: one fp8 DMA (8KB rows)
                # + DVE upcast to bf16 (DVE is otherwise idle)
                sld8 = spool8.tile([P, gw[g] * P], FP8, tag="s8")
                nc.sync.dma_start(sld8[:], sall[:, sw : sw + gw[g] * P])
                sld = spool.tile([P, gw[g] * P], BF16, tag="s")
                nc.vector.tensor_copy(sld[:], sld8[:])
                st = 0
                for j in js:
                    jj = j - g * JG
                    tj = int(Tj[j])
                    tlo = int(Tlo_tot[j])
                    thi = tj - tlo

                    def gath(out_ap, in_ap, idxs_ap, n):
                        nc.gpsimd.dma_gather(
                            out_ap=out_ap, in_ap=in_ap, idxs_ap=idxs_ap,
                            num_idxs=n, num_idxs_reg=n, elem_size=P,
                            single_packet=False, queue_num=next_q(),
                        )

                    G = gpool.tile([P, tj, P], BF16, tag="g")
                    for t0 in range(0, tlo, MAXT):
                        t1 = min(t0 + MAXT, tlo)
                        gath(G[:, t0:t1, :], xlo[:],
                             main_sb[:, iw + t0 * 8 : iw + t1 * 8], (t1 - t0) * P)
                    for t0 in range(0, thi, MAXT):
                        t1 = min(t0 + MAXT, thi)
                        gath(G[:, tlo + t0 : tlo + t1, :], xhi[:],
                             main_sb[:, iw + (tlo + t0) * 8 : iw + (tlo + t1) * 8],
                             (t1 - t0) * P)

                    bps = pswin.tile([P, R, P], F32, tag="psw")
                    for r in range(R):
                        blocks = [int(lo_off[j, r]) + t for t in range(int(T_lo[j, r]))] + \
                                 [int(hi_off[j, r]) + t for t in range(int(T_hi[j, r]))]
                        for k, b in enumerate(blocks):
                            nc.tensor.matmul(
                                bps[:, r, :], lhsT=G[:, b, :],
                                rhs=sld[:, (st + b) * P : (st + b + 1) * P],
                                start=(k == 0), stop=(k == len(blocks) - 1),
                            )
                    nc.scalar.copy(acc4[:, 0:R, jj * P : (jj + 1) * P], bps[:])

                    iw += tj * 8
                    st += tj

                sw += gw[g] * P
                ops = psout.tile([P, JG * P], F32, tag="pso")
                for r in range(NR):
                    nc.tensor.matmul(
                        ops[:], lhsT=wcat_sb[:, r * P : (r + 1) * P],
                        rhs=acc4[:, r, :],
                        start=(r == 0), stop=(r == NR - 1),
                    )
                osb = opool.tile([P, JG * P], BF16, tag="o")
                nc.scalar.activation(
                    osb[:], ops[:], mybir.ActivationFunctionType.Relu,
                    bias=bias_sb[:, 0:1],
                )
                nc.sync.dma_start(out_t[g], osb[:])

    nc.compile()
    return nc


def _host_inputs(inputs):
    x = np.ascontiguousarray(np.asarray(inputs["x"]), dtype=np.float32)
    layout, per_core = _preprocess(np.asarray(inputs["edge_index"]),
                                   np.asarray(inputs["edge_type"]))
    dev = _build_device_arrays(layout, per_core, x)

    weight = np.asarray(inputs["weight"], np.float32)
    root = np.asarray(inputs["root"], np.float32)
    wcat = np.ascontiguousarray(
        np.concatenate([weight[r] for r in range(R)] + [root], axis=1)
    ).astype(BF)
    biasc = np.ascontiguousarray(np.asarray(inputs["bias"], np.float32)[:, None])
    x_bf = x.astype(BF)
    in_maps = [
        {"xlo": np.ascontiguousarray(x_bf[:H]), "xhi": np.ascontiguousarray(x_bf[H:]),
         "xselfT": dev[c]["xselfT"], "main": dev[c]["main"], "sall": dev[c]["sall"],
         "wcat": wcat, "biasc": biasc}
        for c in range(NCORES)
    ]
    return layout, in_maps


def kernel(x, edge_index, edge_type, weight, root, bias, _trace=False):
    inputs = {"x": x, "edge_index": edge_index, "edge_type": edge_type,
              "weight": weight, "root": root, "bias": bias}
    layout, in_maps = _host_inputs(inputs)
    nc = _build_bass(layout)
    res = run_bass_kernel_spmd(nc, in_maps, core_ids=list(range(NCORES)), trace=_trace)

    outs = []
    for c in range(NCORES):
        o = res.results[c]["out_t"].transpose(0, 2, 1).reshape(NG * JG * P, D)
        outs.append(o[:NC_NODES])
    full = np.ascontiguousarray(np.concatenate(outs, axis=0), dtype=np.float32)
    if _trace:
        return full, res
    return full


# revision 37
# speedup vs baseline: 1.2780x; 1.0221x over previous
"""RGCN (relational GCN) layer on 8 Trainium2 NeuronCores.

out = relu(sum_r mean_{e: rel=r, dst=n} x[src_e] @ W_r + x @ root + bias)

Strategy: dst-node sharding (no collectives). Core c owns dst nodes
[c*6250, (c+1)*6250); every edge lives on its dst's owner core, so each core
computes its output slice independently and the host concatenates.

Device algorithm per core, per dst-tile j (128 dst nodes):
  - dma_gather x[src] rows (bf16) for all edges into G [128, T, 128]
    (x is split in two 25000-row halves so gather indices fit int16).
    Gathers rotate across 4 SWDGE queues (single_packet=False) so the
    descriptor rings never back-pressure the Q7 — the drain runs at the
    random-read rate of all 16 SDMA engines instead of one ring. The
    gather drain (~75K random 256B HBM reads per core, DRAM-row-cycle
    bound) is the kernel's critical resource; everything else overlaps.
  - per relation window r: bps[:, r, :] += G_t^T @ S_t over the window's
    tiles, where S_t[p, q] = w_p * (q == col_p) (col = dst_local,
    w = 1/count) -> the per-(r, dst) *mean*. S tiles are precomputed
    dense on the host, stored fp8 (exact for pow-2 counts, <=4% else),
    streamed per 4-tile group and upcast to bf16 on the otherwise-idle
    DVE. (Building S on DVE from metadata contends with the Q7
    descriptor-ring writes on the shared SBUF port; streaming it bf16
    doubles the stream's SDMA-engine time.)
  - the self/root term needs x_j^T as a transform rhs: host supplies it
    directly as a transposed xselfT tensor, one [128, 512] DMA per group
    into the acc4 slice (no gather, no scatter matmul).
  - one ACT copy bps -> acc4 slice (bf16, [feat, r, jj*128] layout).
  - per group of 4 dst-tiles: out^T = sum_r W_r^T @ acc4_r via 9
    PSUM-accumulated matmuls with N=512 (lhsT = wcat bf16 [d, e]).
  - relu + bias in one ACT op (bias per-partition in transposed layout);
    DMA out in bf16. Host transposes each [e, 4*128] group back to [n, e].

All index preprocessing happens on the host; per-(window, half) tile counts
are maxed over the 8 cores so all cores run the same program (SPMD).
"""
import ml_dtypes
import numpy as np

import concourse.bass as bass
import concourse.mybir as mybir
import concourse.tile as tile
from concourse import bacc
from concourse.bass_utils import run_bass_kernel_spmd

N = 50000
E = 600000
D = 128
R = 8
P = 128
NCORES = 8
NC_NODES = N // NCORES          # 6250
NT = (NC_NODES + P - 1) // P    # 49
H = 25000                       # x half split (int16 index limit)
NR = R + 1                      # relations + self
MAXT = 8                        # <= 1024 idxs per dma_gather call
JG = 4                          # dst-tiles per transform group
NG = (NT + JG - 1) // JG        # 13 groups (last has 1 tile)

F32 = mybir.dt.float32
BF16 = mybir.dt.bfloat16
FP8 = mybir.dt.float8e4
I16 = mybir.dt.int16
BF = ml_dtypes.bfloat16
F8 = ml_dtypes.float8_e4m3


def _preprocess(edge_index, edge_type):
    """Core-invariant tile layout + per-core slot arrays (index data only).

    Slot layout per dst-tile j: [lo tiles (windows r=0..7) | hi tiles].
    """
    src = np.asarray(edge_index[0], dtype=np.int64)
    dst = np.asarray(edge_index[1], dtype=np.int64)
    et = np.asarray(edge_type, dtype=np.int64)

    counts = np.bincount(et * N + dst, minlength=R * N)

    core = dst // NC_NODES
    j = (dst - core * NC_NODES) // P
    half = (src >= H).astype(np.int64)

    key = ((core * NT + j) * R + et) * 2 + half
    cnt = np.bincount(key, minlength=NCORES * NT * R * 2).reshape(NCORES, NT, R, 2)
    tiles = -(-cnt // P)
    Tmax = tiles.max(axis=0)                   # [NT, R, 2]

    T_lo = Tmax[:, :, 0].copy()                # [NT, R]
    T_hi = Tmax[:, :, 1].copy()
    both0 = (T_lo + T_hi) == 0
    T_lo[both0] = 1

    Tlo_tot = T_lo.sum(axis=1)
    Thi_tot = T_hi.sum(axis=1)
    Tj = Tlo_tot + Thi_tot                     # gathered tiles (self separate)

    lo_off = np.zeros((NT, R), dtype=np.int64)
    lo_off[:, 1:] = np.cumsum(T_lo, axis=1)[:, :-1]
    hi_off = np.zeros((NT, R), dtype=np.int64)
    hi_off[:, 1:] = np.cumsum(T_hi, axis=1)[:, :-1]
    hi_off += Tlo_tot[:, None]

    S_tiles = int(Tj.sum())

    jkey = core * NT + j
    order = np.lexsort((half, et, jkey))
    src_s, et_s, core_s, half_s = src[order], et[order], core[order], half[order]
    dst_s = dst[order]
    j_s = (dst_s - core_s * NC_NODES) // P
    col_s = (dst_s - core_s * NC_NODES) % P
    w_s = (1.0 / np.maximum(counts[et_s * N + dst_s], 1)).astype(np.float32)

    tile_base = np.zeros(NT, dtype=np.int64)
    tile_base[1:] = np.cumsum(Tj)[:-1]

    per_core = []
    for c in range(NCORES):
        m = core_s == c
        cs, cj, cr, ccol, cw, chalf = (a[m] for a in (src_s, j_s, et_s, col_s, w_s, half_s))
        gidx = np.zeros(S_tiles * P, dtype=np.int32)
        colv = np.zeros(S_tiles * P, dtype=np.float32)
        wv = np.zeros(S_tiles * P, dtype=np.float32)

        if len(cj):
            wkey = (cj * R + cr) * 2 + chalf
            changed = np.empty(len(wkey), dtype=bool)
            changed[0] = True
            changed[1:] = wkey[1:] != wkey[:-1]
            grp_start = np.maximum.accumulate(np.where(changed, np.arange(len(wkey)), 0))
            pos = np.arange(len(wkey)) - grp_start
            block = np.where(chalf == 0, lo_off[cj, cr], hi_off[cj, cr])
            slot = (tile_base[cj] + block + pos // P) * P + (pos % P)
            gidx[slot] = np.where(chalf == 0, cs, cs - H)
            colv[slot] = ccol
            wv[slot] = cw

        per_core.append({"gidx": gidx, "col": colv, "w": wv})

    layout = {
        "T_lo": T_lo, "T_hi": T_hi, "Tlo_tot": Tlo_tot, "Thi_tot": Thi_tot,
        "Tj": Tj, "lo_off": lo_off, "hi_off": hi_off, "S_tiles": S_tiles,
    }
    return layout, per_core


def _wrap_idxs(flat):
    """dma_gather int16 index layout: idx i at [i%16, i//16], replicated x8."""
    a = np.asarray(flat, dtype=np.int16).reshape(-1, 16).T
    return np.tile(a, (8, 1))


def _build_device_arrays(layout, per_core, x):
    """Per-core arrays: main [128, sum(Tj*8)] int16 idx, sall [128,
    sum(Tj)*128] bf16 dense scatter matrices, and xself [NT*128, 128] bf16
    (the core's dst rows)."""
    Tj, S_tiles = layout["Tj"], layout["S_tiles"]
    x_bf = np.asarray(x, dtype=np.float32).astype(BF)
    out = []
    for c, meta in enumerate(per_core):
        gidx, colv, wv = meta["gidx"], meta["col"], meta["w"]
        cols = []
        base = 0
        for j in range(NT):
            tj = int(Tj[j])
            sl = slice(base * P, (base + tj) * P)
            cols.append(_wrap_idxs(gidx[sl]))
            base += tj
        # dense S tiles: S[t, p, col[t*P+p]] = w[t*P+p] (fp8: 1/count values
        # quantize to <=4% err on the ~19% of counts not a power of two;
        # contributes ~1e-3 to the output rel err)
        sall = np.zeros((S_tiles, P, P), dtype=F8)
        ti = np.repeat(np.arange(S_tiles), P)
        pi = np.tile(np.arange(P), S_tiles)
        sall[ti, pi, colv.astype(np.int64)] = wv.astype(F8)
        sall = np.ascontiguousarray(sall.transpose(1, 0, 2).reshape(P, S_tiles * P))
        # transposed self rows [feat, dst_local] for the root-term rhs
        xsT = np.zeros((P, NG * JG * P), dtype=BF)
        n0 = c * NC_NODES
        xsT[:, :NC_NODES] = x_bf[n0 : n0 + NC_NODES].T
        out.append({
            "main": np.ascontiguousarray(np.concatenate(cols, axis=1), dtype=np.int16),
            "sall": sall,
            "xselfT": xsT,
        })
    return out


def _build_bass(layout):
    T_lo, T_hi = layout["T_lo"], layout["T_hi"]
    Tj, Tlo_tot = layout["Tj"], layout["Tlo_tot"]
    lo_off, hi_off = layout["lo_off"], layout["hi_off"]
    S_tiles = layout["S_tiles"]
    IW = int(sum(int(Tj[j]) * 8 for j in range(NT)))

    nc = bacc.Bacc(None, target_bir_lowering=False, debug=False,
                   num_swdge_queues=4, dynamic_dma_scratch_size=65536)

    xlo = nc.dram_tensor("xlo", [H, D], BF16, kind="ExternalInput")
    xhi = nc.dram_tensor("xhi", [H, D], BF16, kind="ExternalInput")
    xselfT = nc.dram_tensor("xselfT", [P, NG * JG * P], BF16, kind="ExternalInput")
    main = nc.dram_tensor("main", [P, IW], I16, kind="ExternalInput")
    sall = nc.dram_tensor("sall", [P, S_tiles * P], FP8, kind="ExternalInput")
    wcat = nc.dram_tensor("wcat", [P, NR * P], BF16, kind="ExternalInput")
    biasc = nc.dram_tensor("biasc", [P, 1], F32, kind="ExternalInput")
    out_t = nc.dram_tensor("out_t", [NG, P, JG * P], F32, kind="ExternalOutput")

    qn = [0]

    def next_q():
        q = qn[0]
        qn[0] = (q + 1) % 4
        return q



    with tile.TileContext(nc) as tc:
        with (
            tc.tile_pool(name="const", bufs=1) as cpool,
            tc.tile_pool(name="g", bufs=6) as gpool,
            tc.tile_pool(name="acc", bufs=2) as apool,
            tc.tile_pool(name="s8", bufs=2) as spool8,
            tc.tile_pool(name="s", bufs=2) as spool,
            tc.tile_pool(name="o", bufs=2) as opool,
            tc.tile_pool(name="psw", bufs=3, space="PSUM") as pswin,
            tc.tile_pool(name="pso", bufs=2, space="PSUM") as psout,
        ):
            wcat_sb = cpool.tile([P, NR * P], BF16)
            nc.sync.dma_start(wcat_sb[:], wcat[:])
            bias_sb = cpool.tile([P, 1], F32)
            nc.sync.dma_start(bias_sb[:], biasc[:])
            # whole int16 gather-index table resident (12.5 KB/partition) —
            # per-group slices would queue behind the big S transfers in the
            # sync HWDGE FIFO and stall the gather issue.
            main_sb = cpool.tile([P, IW], I16)
            nc.sync.dma_start(main_sb[:], main[:])

            gw = [int(sum(int(Tj[j]) for j in range(g * JG, min((g + 1) * JG, NT))))
                  for g in range(NG)]

            iw = 0
            sw = 0
            for g in range(NG):
                js = [j for j in range(g * JG, min((g + 1) * JG, NT))]
                acc4 = apool.tile([P, NR, JG * P], BF16, tag="acc")
                # root-term rhs: transposed self rows, one DMA per group
                nc.scalar.dma_start(acc4[:, R, :],
                                    xselfT[:, g * JG * P : (g + 1) * JG * P])
                # scatter matrices for the whole group: one fp8 DMA (8KB rows)
                # + DVE upcast to bf16 (DVE is otherwise idle)
                sld8 = spool8.tile([P, gw[g] * P], FP8, tag="s8")
                nc.sync.dma_start(sld8[:], sall[:, sw : sw + gw[g] * P])
                sld = spool.tile([P, gw[g] * P], BF16, tag="s")
                nc.vector.tensor_copy(sld[:], sld8[:])
                st = 0
                for j in js:
                    jj = j - g * JG
                    tj = int(Tj[j])
                    tlo = int(Tlo_tot[j])
                    thi = tj - tlo

                    def gath(out_ap, in_ap, idxs_ap, n):
                        nc.gpsimd.dma_gather(
                            out_ap=out_ap, in_ap=in_ap, idxs_ap=idxs_ap,
                            num_idxs=n, num_idxs_reg=n, elem_size=P,
                            single_packet=False, queue_num=next_q(),
                        )

                    G = gpool.tile([P, tj, P], BF16, tag="g")
                    for t0 in range(0, tlo, MAXT):
                        t1 = min(t0 + MAXT, tlo)
                        gath(G[:, t0:t1, :], xlo[:],
                             main_sb[:, iw + t0 * 8 : iw + t1 * 8], (t1 - t0) * P)
                    for t0 in range(0, thi, MAXT):
                        t1 = min(t0 + MAXT, thi)
                        gath(G[:, tlo + t0 : tlo + t1, :], xhi[:],
                             main_sb[:, iw + (tlo + t0) * 8 : iw + (tlo + t1) * 8],
                             (t1 - t0) * P)

                    bps = pswin.tile([P, R, P], F32, tag="psw")
                    for r in range(R):
                        blocks = [int(lo_off[j, r]) + t for t in range(int(T_lo[j, r]))] + \
                                 [int(hi_off[j, r]) + t for t in range(int(T_hi[j, r]))]
                        for k, b in enumerate(blocks):
                            nc.tensor.matmul(
                                bps[:, r, :], lhsT=G[:, b, :],
                                rhs=sld[:, (st + b) * P : (st + b + 1) * P],
                                start=(k == 0), stop=(k == len(blocks) - 1),
                            )
                    nc.scalar.copy(acc4[:, 0:R, jj * P : (jj + 1) * P], bps[:])

                    iw += tj * 8
                    st += tj

                sw += gw[g] * P
                ops = psout.tile([P, JG * P], F32, tag="pso")
                for r in range(NR):
                    nc.tensor.matmul(
                        ops[:], lhsT=wcat_sb[:, r * P : (r + 1) * P],
                        rhs=acc4[:, r, :],
                        start=(r == 0), stop=(r == NR - 1),
                    )
                osb = opool.tile([P, JG * P], F32, tag="o")
                nc.scalar.activation(
                    osb[:], ops[:], mybir.ActivationFunctionType.Relu,
                    bias=bias_sb[:, 0:1],
                )
                nc.sync.dma_start(out_t[g], osb[:])

    nc.compile()
    return nc


def _host_inputs(inputs):
    x = np.ascontiguousarray(np.asarray(inputs["x"]), dtype=np.float32)
    layout, per_core = _preprocess(np.asarray(inputs["edge_index"]),
                                   np.asarray(inputs["edge_type"]))
    dev = _build_device_arrays(layout, per_core, x)

    weight = np.asarray(inputs["weight"], np.float32)
    root = np.asarray(inputs["root"], np.float32)
    wcat = np.ascontiguousarray(
        np.concatenate([weight[r] for r in range(R)] + [root], axis=1)
    ).astype(BF)
    biasc = np.ascontiguousarray(np.asarray(inputs["bias"], np.float32)[:, None])
    x_bf = x.astype(BF)
    in_maps = [
        {"xlo": np.ascontiguousarray(x_bf[:H]), "xhi": np.ascontiguousarray(x_bf[H:]),
         "xselfT": dev[c]["xselfT"], "main": dev[c]["main"], "sall": dev[c]["sall"],
         "wcat": wcat, "biasc": biasc}
        for c in range(NCORES)
    ]
    return layout, in_maps


def kernel(x, edge_index, edge_type, weight, root, bias, _trace=False):
    inputs = {"x": x, "edge_index": edge_index, "edge_type": edge_type,
              "weight": weight, "root": root, "bias": bias}
    layout, in_maps = _host_inputs(inputs)
    nc = _build_bass(layout)
    res = run_bass_kernel_spmd(nc, in_maps, core_ids=list(range(NCORES)), trace=_trace)

    outs = []
    for c in range(NCORES):
        o = res.results[c]["out_t"].transpose(0, 2, 1).reshape(NG * JG * P, D)
        outs.append(o[:NC_NODES])
    full = np.ascontiguousarray(np.concatenate(outs, axis=0), dtype=np.float32)
    if _trace:
        return full, res
    return full
